# revision 10
# baseline (speedup 1.0000x reference)
"""Feature-pyramid ROIAlign (multi-level crop) on 8 TRN2 NeuronCores — v4.

Host routes (level assignment, 11x11 windows, bf16 bilinear weight
matrices); the device gathers each proposal's [11,11,256] bf16 patch from
a channels-last feature arena into a cell-on-partition SBUF tile and
interpolates with two k=121 bf16 matmuls into PSUM, then writes the
c-major f32 output shard.

Patch gathers are split across three engines to spread issue cost:
SP/ACT issue HWDGE DMAs with register-sourced dynamic offsets; GpSimd
issues per-proposal indirect DMAs (one int32 cell-row index per
partition). All 8 cores share one SPMD graph; per-core differences are
pure data (offset tables, weights).
"""
import os
import numpy as np
import ml_dtypes

RPN_SCALES = (2.0, 4.0, 8.0, 16.0)
BASE_SIZES = (8.0, 16.0, 32.0, 64.0)
S = 14
S2 = S * S
PW = 11
CELLS = PW * PW
C = 256
MAP_HW = (256, 128, 64, 32)
ARENA_BASE = (0, 65536, 81920, 86016)  # cell-row base of each level
ARENA_ROWS = 87040
N_CORES = 8
GRP = 8
# per-group engine pattern: 0=SP(dyn), 1=ACT(dyn), 2=GpSimd(indirect)
ENG_PATTERN = (2, 1, 0, 2, 1, 2, 2, 0)

LAST_EXEC_TIME_NS = None
_GRAPH_CACHE = {}


def _route_and_weights(proposals):
    p = proposals.astype(np.float32)
    x0, y0, x1, y1 = p[:, 1], p[:, 2], p[:, 3], p[:, 4]
    sizes = np.sqrt((x1 - x0) * (y1 - y0))
    base = np.asarray(BASE_SIZES, dtype=np.float32)
    dist = np.abs(sizes[:, None] - base[None, :])
    lvl = np.argmin(dist, axis=1).astype(np.int32)

    N = p.shape[0]
    stride = np.asarray(RPN_SCALES, dtype=np.float32)[lvl]
    M = np.asarray(MAP_HW, dtype=np.int32)[lvl]

    fx0, fy0, fx1, fy1 = (c / stride for c in (x0, y0, x1, y1))
    bw = (fx1 - fx0) / np.float32(S)
    bh = (fy1 - fy0) / np.float32(S)
    grid = np.arange(S, dtype=np.float32) + np.float32(0.5)
    xs = fx0[:, None] + grid[None, :] * bw[:, None] - np.float32(0.5)
    ys = fy0[:, None] + grid[None, :] * bh[:, None] - np.float32(0.5)

    def split(coord, Mv):
        c0 = np.floor(coord)
        frac = coord - c0
        i0 = np.clip(c0.astype(np.int64), 0, Mv - 1).astype(np.int32)
        i1 = np.minimum(i0 + 1, Mv - 1).astype(np.int32)
        return i0, i1, frac.astype(np.float32)

    Mv = M[:, None]
    yi0, yi1, wy = split(ys, Mv)
    xi0, xi1, wx = split(xs, Mv)

    oy = np.clip(yi0.min(axis=1), 0, M - PW)
    ox = np.clip(xi0.min(axis=1), 0, M - PW)
    ly0, ly1 = yi0 - oy[:, None], yi1 - oy[:, None]
    lx0, lx1 = xi0 - ox[:, None], xi1 - ox[:, None]
    assert ly0.min() >= 0 and lx0.min() >= 0 and ly1.max() < PW and lx1.max() < PW, \
        "proposal spans >11 feature cells; patch window too small"

    ii = np.arange(S)
    nn = np.arange(N)[:, None]
    Wy = np.zeros((N, S, PW), dtype=np.float32)
    Wx = np.zeros((N, S, PW), dtype=np.float32)
    np.add.at(Wy, (nn, ii[None, :], ly0), 1.0 - wy)
    np.add.at(Wy, (nn, ii[None, :], ly1), wy)
    np.add.at(Wx, (nn, ii[None, :], lx0), 1.0 - wx)
    np.add.at(Wx, (nn, ii[None, :], lx1), wx)
    Wfull = np.einsum("niy,njx->nyxij", Wy, Wx).reshape(N, CELLS, S2)
    return lvl, oy.astype(np.int32), ox.astype(np.int32), Wfull


def _shard(lvl):
    slot_gid = [[] for _ in range(N_CORES)]
    level_seq = []
    for l in range(4):
        ids = np.where(lvl == l)[0]
        if len(ids) == 0:
            continue
        pad = (-len(ids)) % N_CORES
        ids = np.concatenate([ids, np.repeat(ids[-1], pad)])
        per = len(ids) // N_CORES
        for k in range(N_CORES):
            slot_gid[k].extend(ids[k::N_CORES].tolist())
        level_seq.extend([l] * per)
    return (np.asarray(slot_gid, dtype=np.int64),
            np.asarray(level_seq, dtype=np.int64))


def _slot_engines(M):
    """Engine id per slot, and per-engine orderings."""
    eng = [ENG_PATTERN[j - (j // GRP) * GRP] for j in range(M)]
    sp = [j for j in range(M) if eng[j] == 0]
    act = [j for j in range(M) if eng[j] == 1]
    q7 = [j for j in range(M) if eng[j] == 2]
    return np.asarray(eng), sp, act, q7


def _build_graph(level_seq):
    import concourse.bass as bass
    import concourse.bacc as bacc
    import concourse.mybir as mybir
    import concourse.tile as tile

    M = len(level_seq)
    eng, sp_slots, act_slots, q7_slots = _slot_engines(M)
    n_sp, n_act, n_q7 = len(sp_slots), len(act_slots), len(q7_slots)
    sp_pos = {j: i for i, j in enumerate(sp_slots)}
    act_pos = {j: i for i, j in enumerate(act_slots)}
    q7_pos = {j: i for i, j in enumerate(q7_slots)}

    SP = (mybir.EngineType.SP,)
    ACT = (mybir.EngineType.Activation,)
    nc = bacc.Bacc()
    arena = nc.declare_dram_parameter("arena", [ARENA_ROWS, C],
                                      mybir.dt.bfloat16, isOutput=False)
    lvl_view = [arena[ARENA_BASE[l]:ARENA_BASE[l] + MAP_HW[l] * MAP_HW[l], :]
                for l in range(4)]
    wmat = nc.declare_dram_parameter("wmat", [CELLS, M, S2], mybir.dt.bfloat16,
                                     isOutput=False)
    # fused (oy*W+ox) offsets for SP slots then ACT slots (level-relative)
    orig = nc.declare_dram_parameter("orig", [1, max(n_sp + n_act, 1)],
                                     mybir.dt.int32, isOutput=False)
    # absolute arena cell ids for Q7 slots, [121, n_q7]
    idxg = nc.declare_dram_parameter("idxg", [CELLS, max(n_q7, 1)],
                                     mybir.dt.int32, isOutput=False)
    out = nc.declare_dram_parameter("out", [C, M, S2], mybir.dt.bfloat16,
                                    isOutput=True)

    n_groups = (M + GRP - 1) // GRP
    with tile.TileContext(nc) as tc:
        with (
            tc.tile_pool(name="small", bufs=1) as psmall,
            tc.tile_pool(name="wpool", bufs=4) as pwp,
            tc.tile_pool(name="patch", bufs=32) as pp,
            tc.tile_pool(name="outp", bufs=4) as po,
            tc.tile_pool(name="ps", bufs=4, space="PSUM") as ppsum,
        ):
            orig_t = psmall.tile([1, max(n_sp + n_act, 1)], mybir.dt.int32)
            nc.sync.dma_start(orig_t[:], orig[:])
            idxg_t = psmall.tile([CELLS, max(n_q7, 1)], mybir.dt.int32)
            nc.sync.dma_start(idxg_t[:], idxg[:])
            sp_used = act_used = 0
            for g in range(n_groups):
                a = g * GRP
                b = min(a + GRP, M)
                grp = b - a
                wt = pwp.tile([CELLS, grp * S2], mybir.dt.bfloat16, tag="wt")
                nc.sync.dma_start(
                    wt[:].rearrange("k (p n) -> k p n", p=grp),
                    wmat[:, a:b, :])
                # batched register loads for this group's SP/ACT slots
                g_sp = [j for j in range(a, b) if eng[j] == 0]
                g_act = [j for j in range(a, b) if eng[j] == 1]
                vals_sp = vals_act = ()
                if g_sp:
                    o = sp_pos[g_sp[0]]
                    _, vals_sp = nc.values_load_multi_w_load_instructions(
                        orig_t[0:1, o:o + len(g_sp)], engines=SP,
                        skip_runtime_bounds_check=True)
                if g_act:
                    o = n_sp + act_pos[g_act[0]]
                    _, vals_act = nc.values_load_multi_w_load_instructions(
                        orig_t[0:1, o:o + len(g_act)], engines=ACT,
                        skip_runtime_bounds_check=True)
                pts = []
                for q in range(grp):
                    j = a + q
                    l = level_seq[j]
                    Wl = MAP_HW[l]
                    pt = pp.tile([CELLS, C], mybir.dt.bfloat16, tag="pt")
                    if eng[j] == 2:
                        nc.gpsimd.indirect_dma_start(
                            out=pt[:],
                            out_offset=None,
                            in_=arena[:],
                            in_offset=bass.IndirectOffsetOnAxis(
                                ap=idxg_t[:, q7_pos[j]:q7_pos[j] + 1], axis=0),
                        )
                    else:
                        if eng[j] == 0:
                            e = nc.sync
                            comb = vals_sp[g_sp.index(j)]
                        else:
                            e = nc.scalar
                            comb = vals_act[g_act.index(j)]
                        src = (lvl_view[l][bass.ds(comb, PW * Wl), :]
                               .rearrange("(y x) c -> y x c", x=Wl)[:, 0:PW, :])
                        e.dma_start(pt[:], src)
                    pts.append(pt)
                outA = po.tile([128, grp * S2], mybir.dt.bfloat16, tag="outA")
                outB = po.tile([128, grp * S2], mybir.dt.bfloat16, tag="outB")
                for q0 in range(0, grp, 2):
                    pair = min(2, grp - q0)
                    psA = ppsum.tile([128, pair * S2], mybir.dt.float32, tag="psA")
                    psB = ppsum.tile([128, pair * S2], mybir.dt.float32, tag="psB")
                    for dq in range(pair):
                        q = q0 + dq
                        sl_w = slice(q * S2, (q + 1) * S2)
                        sl_p = slice(dq * S2, (dq + 1) * S2)
                        nc.tensor.matmul(psA[:, sl_p], pts[q][:, 0:128],
                                         wt[:, sl_w], start=True, stop=True)
                        nc.tensor.matmul(psB[:, sl_p], pts[q][:, 128:256],
                                         wt[:, sl_w], start=True, stop=True)
                    sl_o = slice(q0 * S2, (q0 + pair) * S2)
                    nc.vector.tensor_copy(outA[:, sl_o], psA[:])
                    nc.vector.tensor_copy(outB[:, sl_o], psB[:])
                nc.sync.dma_start(out[0:128, a:b, :], outA[:])
                nc.scalar.dma_start(out[128:256, a:b, :], outB[:])
    nc.finalize()
    return nc


def _prep_core_inputs(k, slot_gid, level_seq, lvl, oy, ox, Wbf):
    M = slot_gid.shape[1]
    eng, sp_slots, act_slots, q7_slots = _slot_engines(M)
    gids = slot_gid[k]
    wm = np.ascontiguousarray(Wbf[gids].transpose(1, 0, 2))  # [121, M, 196]

    Wl = np.asarray(MAP_HW)[lvl[gids]].astype(np.int64)
    comb = oy[gids].astype(np.int64) * Wl + ox[gids]  # level-relative
    og = np.concatenate([comb[sp_slots], comb[act_slots]])
    og = np.ascontiguousarray(og.reshape(1, -1).astype(np.int32))
    if og.size == 0:
        og = np.zeros((1, 1), np.int32)

    dy = np.repeat(np.arange(PW), PW)
    dx = np.tile(np.arange(PW), PW)
    ig = np.zeros((CELLS, max(len(q7_slots), 1)), np.int64)
    for i, j in enumerate(q7_slots):
        g = gids[j]
        W = MAP_HW[lvl[g]]
        ig[:, i] = (ARENA_BASE[lvl[g]] + (oy[g] + dy) * W + (ox[g] + dx))
    idxg = np.ascontiguousarray(ig.astype(np.int32))
    return wm, og, idxg


def _install_profile_hook():
    """Register the NTFF profile hook (ctypes into libaxon_pjrt.so) so
    run_bass_kernel_spmd(trace=True) can report exec_time_ns under axon.
    No-op if already present or the .so lacks the symbols."""
    import contextlib
    import ctypes
    import sys
    import types
    if "antenv.axon_hooks" in sys.modules:
        return
    so_path = "/opt/axon/libaxon_pjrt.so"
    try:
        lib = ctypes.CDLL(so_path)
        lib.axon_start_nrt_profile.argtypes = [
            ctypes.POINTER(ctypes.c_int64), ctypes.c_size_t]
        lib.axon_start_nrt_profile.restype = ctypes.c_int64
        lib.axon_stop_nrt_profile.argtypes = [ctypes.c_char_p]
        lib.axon_stop_nrt_profile.restype = ctypes.c_int64
    except (OSError, AttributeError):
        return

    @contextlib.contextmanager
    def _hook(output_dir, device_ids):
        import jax
        jax.devices()
        if device_ids:
            ids = (ctypes.c_int64 * len(device_ids))(*device_ids)
            rc = lib.axon_start_nrt_profile(ids, len(device_ids))
        else:
            rc = lib.axon_start_nrt_profile(None, 0)
        if rc != 0:
            raise RuntimeError(f"axon_start_nrt_profile rc={rc}")
        try:
            yield
        finally:
            n = lib.axon_stop_nrt_profile(str(output_dir).encode())
            if n < 0:
                raise RuntimeError(f"axon_stop_nrt_profile rc={n}")

    mod = types.ModuleType("antenv.axon_hooks")
    mod.get_axon_ntff_profile_hook = lambda: _hook
    mod.set_axon_ntff_profile_hook = lambda h: None
    sys.modules["antenv.axon_hooks"] = mod
    try:
        import antenv
        antenv.axon_hooks = mod
    except ImportError:
        pass


def kernel(f0, f1, f2, f3, proposals):
    global LAST_EXEC_TIME_NS
    try:
        _install_profile_hook()
    except Exception:
        pass
    from concourse.bass_utils import run_bass_kernel_spmd

    feats = (f0, f1, f2, f3)
    N = proposals.shape[0]
    lvl, oy, ox, Wfull = _route_and_weights(np.asarray(proposals))
    slot_gid, level_seq = _shard(lvl)
    M = slot_gid.shape[1]

    key = tuple(level_seq.tolist())
    if key not in _GRAPH_CACHE:
        _GRAPH_CACHE[key] = _build_graph(level_seq)
    nc = _GRAPH_CACHE[key]

    arena_np = np.concatenate([
        np.ascontiguousarray(np.asarray(f)[0].transpose(1, 2, 0)).astype(
            ml_dtypes.bfloat16).reshape(-1, C)
        for f in feats
    ], axis=0)
    assert arena_np.shape[0] == ARENA_ROWS
    Wbf = Wfull.astype(ml_dtypes.bfloat16)

    in_maps = []
    for k in range(N_CORES):
        wm, og, idxg = _prep_core_inputs(k, slot_gid, level_seq, lvl, oy, ox, Wbf)
        in_maps.append({"arena": arena_np, "wmat": wm, "orig": og, "idxg": idxg})

    trace = os.environ.get("KERNEL_TRACE", "0") == "1"
    res = run_bass_kernel_spmd(nc, in_maps, list(range(N_CORES)), trace=trace)
    LAST_EXEC_TIME_NS = res.exec_time_ns

    out_full = np.zeros((N, C, S2), dtype=np.float32)
    for k in range(N_CORES):
        out_full[slot_gid[k]] = res.results[k]["out"].astype(np.float32).transpose(1, 0, 2)
    return out_full.reshape(N, C, S, S)
